# revision 3
# baseline (speedup 1.0000x reference)
"""Bass/Tile TRN2 kernel for nn_Attention_12489764897521.

attns[b, n] = sum_h W[0, h] * tanh(decoder[b, h] + static[b, h, n] + dynamic[b, h, n])

Full shapes: static/dynamic [32, 256, 10000] f32, decoder [32, 256] f32,
W [1, 256] f32 -> attns [32, 10000] f32.

Sharding: data-parallel over batch B across 8 cores (4 batches/core); W
replicated. The kernel is HBM-bandwidth-bound: 82 MB of input per core at
the ~270-410 GB/s/core HBM share (each NC pair splits one stack; the device
is shared, so tenancy moves the rate run-to-run). The design goal is a
gapless DMA stream plus minimal fixed overhead around it:

  - Engine dispatch is IN-ORDER per engine, so loads are only issued by
    engines that run nothing else: static loads on the SP HWDGE ring,
    dynamic loads on the gpsimd SWDGE queue, stores on the ACT HWDGE ring
    (reached right after ACT's own copies). W/decoder preloads ride gpsimd
    behind item 0's dynamic load; they're first consumed ~15 us in.
  - Uniform 2500-wide n-chunks, each a fused 3D-AP load of both H-halves
    [128, 2*2500] (2.56 MB/transfer); s/d pools 3-deep so the rings always
    have a queued transfer.
  - Per chunk: DVE adds s+d per half -> bf16 tiles; ACT tanh (the decoder
    column rides the activation bias port) -> bf16 tiles; PE contracts the
    two H-halves into psum [1, <=500] slices (bf16 matmul, 1 cyc/row, one
    LDWEIGHTS per W column per chunk); single-lane psum->SBUF copies all
    ride DVE — ACT's in-order queue must never embed an op that waits on
    PE, or that lag compounds item-over-item into an end-of-stream drain;
    one store per chunk via a double-buffered stage row.
  - The matmul/copy/store stage runs one chunk BEHIND add/tanh, so no
    engine's queue waits on a dependency newer than one chunk old and the
    chunks pipeline at engine-busy pace rather than chain-latency pace.
  - Tail: the last batch ends 1000, 800, 700 wide (each chain hides under
    the remaining loads). v7 drain rework: all three tail chunks' loads
    issue BEFORE any tail compute (gpsimd's queue never has an add ahead
    of a d-load emission), tail adds split engines (h0 on DVE, h1 on the
    by-then-idle gpsimd), and the last 2500 chunk's store joins the SP
    flush so end-of-stream ACT is pure tanh. Fixed costs that remain: ~4
    us engine boot, ~9 us semaphore-reset epilogue (framework-emitted,
    same for any TileContext kernel).

Optimization attempts that did NOT beat this design (measured with
interleaved A/B reps; run-to-run tenancy/phase noise is +-10 %, 252-324
us for identical code, so compare medians over >=6 reps only):
  - Longer HBM rows via H-half-split loads (20 KB rows) or full-row
    (b, t) items with cast-on-load bf16 dynamic + in-place adds (40 KB
    rows): the saturated steady-state rate stays ~307 GB/s/NC either
    way — the wall is shared-stack HBM supply, not descriptor overhead.
    Best draws already stream at the ~358 GB/s/NC cap in THIS design.
  - Finer tail taper (900/700/500/400), fewer pool slots (a/t 6->4),
    singles preloads on the ACT ring: all at-or-worse within noise.
"""

from contextlib import ExitStack

import numpy as np

B, H, N = 32, 256, 10000
N_CORES = 8
B_LOC = B // N_CORES  # 4 batches per core
P = 128
NT = H // P  # 2 H-halves
NC = 2500  # n-chunk width; each load fuses both H-halves -> [128, 2*NC]
JC = 500  # matmul free-dim chunk (<= 512, one PSUM bank)

_cache = {}


def _build():
    import concourse.bacc as bacc
    import concourse.mybir as mybir
    import concourse.tile as tile

    nc = bacc.Bacc(
        "TRN2", target_bir_lowering=False, debug=False, num_devices=N_CORES
    )
    st = nc.dram_tensor(
        "static_hidden", [B_LOC, H, N], mybir.dt.float32, kind="ExternalInput"
    ).ap()
    dy = nc.dram_tensor(
        "dynamic_hidden", [B_LOC, H, N], mybir.dt.float32, kind="ExternalInput"
    ).ap()
    dec = nc.dram_tensor(
        "decoder_hidden", [B_LOC, H], mybir.dt.float32, kind="ExternalInput"
    ).ap()
    w = nc.dram_tensor("W", [1, H], mybir.dt.float32, kind="ExternalInput").ap()
    out = nc.dram_tensor(
        "attns", [B_LOC, N], mybir.dt.float32, kind="ExternalOutput"
    ).ap()

    f32 = mybir.dt.float32
    bf16 = mybir.dt.bfloat16
    with tile.TileContext(nc) as tc, ExitStack() as ctx:
        singles = ctx.enter_context(tc.tile_pool(name="singles", bufs=1))
        s_pool = ctx.enter_context(tc.tile_pool(name="s", bufs=3))
        d_pool = ctx.enter_context(tc.tile_pool(name="d", bufs=3))
        a_pool = ctx.enter_context(tc.tile_pool(name="a", bufs=6))
        t_pool = ctx.enter_context(tc.tile_pool(name="t", bufs=6))
        stage_pool = ctx.enter_context(tc.tile_pool(name="stage", bufs=2))
        psum_pool = ctx.enter_context(
            tc.tile_pool(name="psum", bufs=8, space="PSUM")
        )

        # W as two [128, 1] columns (one per H-half), decoder as [128, 1]
        # bias columns indexed [t * B_LOC + b]. These preloads ride the
        # gpsimd SWDGE queue (behind item 0's dynamic load, so they never
        # delay the first big transfer); they are only consumed by the
        # first tanh, ~15 us in. bf16 matmul inputs run the PE at 1
        # cycle/row; tanh's bf16 in/out tiles halve tanh-tile SBUF. All
        # values are O(1), so the ~2^-9 roundings stay ~100x inside the
        # tolerance gate.
        w_sb = singles.tile([P, NT], f32)
        dec_sb = singles.tile([P, NT * B_LOC], f32)
        w_r = singles.tile([P, NT], bf16)

        def load_singles():
            w_cols = w.rearrange("o (t p) -> t p o", p=P)
            dec_r = dec.rearrange("b (t p) -> t p b", p=P)
            for t in range(NT):
                nc.gpsimd.dma_start(w_sb[:, t : t + 1], w_cols[t])
                nc.gpsimd.dma_start(
                    dec_sb[:, t * B_LOC : (t + 1) * B_LOC], dec_r[t]
                )
            nc.vector.tensor_copy(w_r[:], w_sb[:])

        # DRAM views with the H-halves split out: [b, p, t, n] so one DMA
        # pulls both halves of an n-chunk.
        st_r = st.rearrange("b (t p) n -> b p t n", p=P)
        dy_r = dy.rearrange("b (t p) n -> b p t n", p=P)

        # Uniform 2500-wide chunks; the last batch ends 1250, 625, 625 so
        # the post-final-load serial chain is short.
        work = []
        for b in range(B_LOC - 1):
            work += [(b, j * NC, NC) for j in range(N // NC)]
        work += [(3, 0, 2500), (3, 2500, 2500), (3, 5000, 2500)]
        tail3 = [(3, 7500, 1000), (3, 8500, 800), (3, 9300, 700)]

        # Engine dispatch is in-order per engine, so a dma_start must never
        # be queued behind compute or a not-yet-ready store: static loads
        # ride SP alone (the sync engine runs nothing else), dynamic loads
        # ride the gpsimd SWDGE queue (the Pool engine runs no compute),
        # and stores ride the ACT HWDGE ring — ACT reaches a store right
        # after its own psum copies, so it never blocks a load.
        #
        # The matmul/copy/store stage runs one item BEHIND the add/tanh
        # stage. Without the lag, DVE's queue is [adds(k), copies(k),
        # adds(k+1), ...] and copies(k) wait on matmuls(k) -> tanh(k,1) ->
        # add(k,1): a serial cross-engine cycle (~12 us/item) that gates
        # the next item's adds. With the lag each engine's queue only
        # depends one item back, so items pipeline at engine-busy pace.
        def reduce_item(item, tail=False, defer=None):
            b, n0, ncw, tanh_tiles = item
            if defer is None:
                defer = tail
            # Tail stages get dedicated slots (distinct tags): they stay
            # alive until the post-loop store flush, so they must never
            # share a ring slot with an undispatched store's source.
            # (A defer=True on a non-tail item keeps the shared tag; safe
            # only when no later item allocates from it — the last 2500
            # chunk qualifies.)
            stage = stage_pool.tile(
                [1, ncw], f32, tag=f"ts{n0}" if tail else "stage",
                name="stage", bufs=1 if tail else None,
            )
            nj = (ncw + JC - 1) // JC
            slices = []
            for j in range(nj):
                j0 = j * JC
                jw = min(JC, ncw - j0)
                slices.append((slice(j0, j0 + jw), jw))
            # All w0 (start) matmuls, then all w1 (stop) matmuls: the PE
            # keeps one stationary column across each pass, so 2 LDWEIGHTS
            # per item instead of 2 per psum slice (~2 us/item of PE).
            pts = []
            for t in range(NT):
                for j, (jl, jw) in enumerate(slices):
                    if t == 0:
                        pts.append(
                            psum_pool.tile([1, JC], f32, tag="pt", name="pt")
                        )
                    nc.tensor.matmul(
                        pts[j][:1, :jw], w_r[:, t : t + 1],
                        tanh_tiles[t][:, jl],
                        start=(t == 0), stop=(t == NT - 1),
                    )
            for j, (jl, jw) in enumerate(slices):
                # ALL psum->SBUF copies ride DVE: a copy waits on its PE
                # matmuls, and ACT's in-order queue must never embed an
                # op that waits on PE — that lag compounds item over item
                # (~13 us of ACT backlog by stream end when copies
                # alternated onto ACT). DVE's adds+copies (~8.8 us/item)
                # stay under the per-item load wall (>=12.8 us).
                nc.vector.tensor_copy(stage[:, jl], pts[j][:1, :jw])
            if defer:
                # Deferred stores ride SP, but only AFTER every load:
                # emitted inline they would sit between the last static
                # loads in SP's in-order queue and block them on a compute
                # chain (or, on ACT, embed a PE-wait in its queue).
                tail_stores.append((out[b : b + 1, n0 : n0 + ncw], stage[:]))
            else:
                nc.scalar.dma_start(out[b : b + 1, n0 : n0 + ncw], stage[:])

        prev = None
        tail_stores = []
        for b, n0, ncw in work:
            # Fused load of both H-halves: SBUF [128, 2*ncw], half t in
            # columns [t*ncw, (t+1)*ncw).
            s_t = s_pool.tile([P, NT * ncw], f32, tag="s")
            nc.sync.dma_start(
                s_t[:].rearrange("p (t n) -> p t n", t=NT),
                st_r[b, :, :, n0 : n0 + ncw],
            )
            d_t = d_pool.tile([P, NT * ncw], f32, tag="d")
            nc.gpsimd.dma_start(
                d_t[:].rearrange("p (t n) -> p t n", t=NT),
                dy_r[b, :, :, n0 : n0 + ncw],
            )
            if prev is None:
                load_singles()
            # Per-half adds so tanh(h0) overlaps add(h1) on the two
            # engines; tanh needs one call per half anyway (different
            # per-partition bias column).
            tanh_tiles = []
            for t in range(NT):
                hs = slice(t * ncw, (t + 1) * ncw)
                a_t = a_pool.tile([P, ncw], bf16, tag="a")
                nc.vector.tensor_add(a_t[:], s_t[:, hs], d_t[:, hs])
                t_t = t_pool.tile([P, ncw], bf16, tag="t")
                nc.scalar.activation(
                    t_t[:],
                    a_t[:],
                    mybir.ActivationFunctionType.Tanh,
                    bias=dec_sb[:, t * B_LOC + b : t * B_LOC + b + 1],
                )
                tanh_tiles.append(t_t)
            if prev is not None:
                reduce_item(prev)
            prev = (b, n0, ncw, tanh_tiles)

        # ---- Drain (last 2500 columns of batch 3) ----
        # v1's drain was DVE/ACT-throughput-bound (~13 us): after the last
        # byte, DVE still had 6 tail adds + copies queued while gpsimd sat
        # idle (its SWDGE emissions were done). Three changes:
        #   1. ALL tail loads issue before any tail compute, so gpsimd's
        #      in-order queue never has an add ahead of a d-load emission
        #      (slots line up: each tail chunk takes the slot of a 2500
        #      chunk whose adds completed long before).
        #   2. Tail adds split engines: h0 on DVE, h1 on gpsimd — the two
        #      halves' adds run concurrently at the drain.
        #   3. The last 2500 chunk's store is deferred to the SP flush
        #      (tag stays "stage" — nothing allocates from it afterwards),
        #      so ACT's queue never embeds a wait on PE/DVE at the drain.
        tl = []
        for b, n0, ncw in tail3:
            s_t = s_pool.tile([P, NT * ncw], f32, tag="s")
            nc.sync.dma_start(
                s_t[:].rearrange("p (t n) -> p t n", t=NT),
                st_r[b, :, :, n0 : n0 + ncw],
            )
            d_t = d_pool.tile([P, NT * ncw], f32, tag="d")
            nc.gpsimd.dma_start(
                d_t[:].rearrange("p (t n) -> p t n", t=NT),
                dy_r[b, :, :, n0 : n0 + ncw],
            )
            tl.append((b, n0, ncw, s_t, d_t))
        for b, n0, ncw, s_t, d_t in tl:
            tanh_tiles = []
            for t in range(NT):
                hs = slice(t * ncw, (t + 1) * ncw)
                a_t = a_pool.tile([P, ncw], bf16, tag="a")
                add_eng = nc.vector if t == 0 else nc.gpsimd
                add_eng.tensor_add(a_t[:], s_t[:, hs], d_t[:, hs])
                t_t = t_pool.tile([P, ncw], bf16, tag="t")
                nc.scalar.activation(
                    t_t[:],
                    a_t[:],
                    mybir.ActivationFunctionType.Tanh,
                    bias=dec_sb[:, t * B_LOC + b : t * B_LOC + b + 1],
                )
                tanh_tiles.append(t_t)
            reduce_item(prev, tail=prev[2] < NC, defer=True)
            prev = (b, n0, ncw, tanh_tiles)
        reduce_item(prev, tail=True)
        for out_ap, stage_t in tail_stores:
            nc.sync.dma_start(out_ap, stage_t)

    nc.compile()
    return nc


def _run(inputs, **spmd_kwargs):
    from concourse import bass_utils

    if "nc" not in _cache:
        _cache["nc"] = _build()
    nc = _cache["nc"]

    static_hidden = np.asarray(inputs["static_hidden"], dtype=np.float32)
    dynamic_hidden = np.asarray(inputs["dynamic_hidden"], dtype=np.float32)
    decoder_hidden = np.asarray(inputs["decoder_hidden"], dtype=np.float32)
    W = np.ascontiguousarray(np.asarray(inputs["W"], dtype=np.float32))

    in_maps = []
    for i in range(N_CORES):
        sl = slice(i * B_LOC, (i + 1) * B_LOC)
        in_maps.append(
            {
                "static_hidden": np.ascontiguousarray(static_hidden[sl]),
                "dynamic_hidden": np.ascontiguousarray(dynamic_hidden[sl]),
                "decoder_hidden": np.ascontiguousarray(decoder_hidden[sl]),
                "W": W,
            }
        )
    res = bass_utils.run_bass_kernel_spmd(
        nc, in_maps, core_ids=list(range(N_CORES)), **spmd_kwargs
    )
    out = np.concatenate([r["attns"] for r in res.results], axis=0)
    return out, res


def kernel(**inputs):
    out, _ = _run(inputs)
    return out

